# revision 1
# baseline (speedup 1.0000x reference)
"""Associative-embedding loss kernel for Trainium2, SPMD over 8 NeuronCores.

Inputs (full):
  tags:      [128, 65536, 4] float32
  keypoints: [128, 30, 17, 2] int32  (last dim = (heatmap_idx, valid_flag))
Output: [128, 2] float32 = stack([push, pull], -1)

Strategy: data parallel over the batch dim — 16 batches per core. Only
channel 0 of tags at 30*17 keypoint locations per batch is ever read, so
instead of streaming the 17MB tags shard each core fetches 2x5120 256-byte
rows (the rows containing its gather targets) with the GPSIMD dma_gather
ucode instruction, selects the wanted f32 from each row with a one-hot
compare+reduce on the vector engine, transposes the result once on the PE,
and computes the whole loss with ~40 small vector ops.

Slot layout (per core): global slot j in [0, 10240); c = j//128 indexes a
(batch b = c//5, block bl = c%5) pair, u = j%128 indexes (person-in-block
pidx = u//17 < 7, keypoint k = u%17). Person index pp = bl*7 + pidx; slots
with pp >= 30 or u >= 119 are padding (flag 0). dma_gather lands slot j at
[j%128, j//128] so one PE transpose yields the (c)-partitioned layout the
per-person reductions need. Keypoints are fed to the device as pure host-side
permutations of the original tensor (kpw/kpq/kpf below); iota tables and the
transpose identity are position constants.
"""

import numpy as np

import concourse.bacc as bacc
import concourse.bass as bass
import concourse.mybir as mybir

B, N, D = 128, 65536, 4
P, K = 30, 17
NCORES = 8
BC = B // NCORES          # 16 batches per core
NBLK = 5                  # person blocks per batch (7,7,7,7,2 persons)
NC_ = BC * NBLK           # 80 (b, bl) pairs per core
NSLOT = NC_ * 128         # 10240 gather slots per core
NHALF = NSLOT // 2        # 5120 per dma_gather (one per batch-half)
ES = 64                   # gathered row length (f32) = 256 bytes
PP35 = NBLK * 7           # 35 padded person slots per batch
PAIR = PP35 * PP35        # 1225

f32 = mybir.dt.float32
i32 = mybir.dt.int32
i16 = mybir.dt.int16
Add = mybir.AluOpType.add
Sub = mybir.AluOpType.subtract
Mult = mybir.AluOpType.mult
Max = mybir.AluOpType.max
IsGt = mybir.AluOpType.is_gt
IsLt = mybir.AluOpType.is_lt
IsEq = mybir.AluOpType.is_equal
BAnd = mybir.AluOpType.bitwise_and
Div = mybir.AluOpType.divide
X = mybir.AxisListType.X
XY = mybir.AxisListType.XY


def _slot_tables():
    j = np.arange(NSLOT)
    c = j // 128
    u = j % 128
    b = c // NBLK
    bl = c % NBLK
    pidx = u // K
    k = u % K
    pp = bl * 7 + pidx
    real = (u < 119) & (pp < P)
    return j, c, u, b, np.minimum(pp, P - 1), np.minimum(k, K - 1), real


def host_layouts(kp_core: np.ndarray):
    """Pure permutations of the per-core keypoints tensor (+ zero padding)."""
    j, c, u, b, pp, k, real = _slot_tables()
    idx_j = np.where(real, kp_core[b, pp, k, 0], 0).astype(np.int32)
    flg_j = np.where(real, kp_core[b, pp, k, 1], 0).astype(np.int32)

    # wrapped layout for dma_gather indices: slot j of half h at
    # [j%16, 320h + (j%5120)//16], replicated across the 8 gpsimd cores
    kpw = np.zeros((16, NSLOT // 16), np.int32)
    h = j // NHALF
    jh = j % NHALF
    kpw[jh % 16, (NHALF // 16) * h + jh // 16] = idx_j
    kpw = np.tile(kpw, (8, 1))                      # [128, 640]

    kpq = np.zeros((128, NC_), np.int32)            # gather-output layout
    kpq[u, c] = idx_j

    kpf = np.zeros((NC_, 128), np.int32)            # post-transpose layout
    kpf[c, u] = flg_j
    return kpw, kpq, kpf


# HW's f32->i16 cast rounds to nearest, so bake a -7.5/16 bias into the row
# base so round(idx/16 + bias) == idx >> 4; the simulator truncates instead
# (set to 0.0 for sim runs)
R16_BIAS = -0.46875


def host_consts():
    """Input-independent position tables."""
    f = np.arange(NSLOT // 16)
    bofs = np.tile(((f // 40 % 8) * (N // 16)).astype(np.int32), (128, 1))
    iot16 = np.tile(np.tile(np.arange(16, dtype=np.int32), NC_), (128, 1))  # [128,1280]
    iq = np.tile(np.tile(np.arange(PP35, dtype=np.int32), PP35), (BC, 1))   # [16,1225] p2
    ip = np.tile(np.repeat(np.arange(PP35, dtype=np.int32), PP35), (BC, 1))  # p1
    ident = np.eye(128, dtype=np.float32)
    return bofs, iot16, iq, ip, ident


def build_nc(detect_race_conditions: bool = True) -> bass.Bass:
    nc = bacc.Bacc("TRN2", target_bir_lowering=False, debug=False,
                   num_swdge_queues=4,
                   detect_race_conditions=detect_race_conditions)
    tags = nc.declare_dram_parameter("tags", [BC, N, D], f32, isOutput=False)
    kpw_d = nc.declare_dram_parameter("kpw", [128, NSLOT // 16], i32, isOutput=False)
    kpq_d = nc.declare_dram_parameter("kpq", [128, NC_], i32, isOutput=False)
    kpf_d = nc.declare_dram_parameter("kpf", [NC_, 128], i32, isOutput=False)
    bofs_d = nc.declare_dram_parameter("bofs", [128, NSLOT // 16], i32, isOutput=False)
    iot16_d = nc.declare_dram_parameter("iot16", [128, 16 * NC_], i32, isOutput=False)
    iq_d = nc.declare_dram_parameter("iq", [BC, PAIR], i32, isOutput=False)
    ip_d = nc.declare_dram_parameter("ip", [BC, PAIR], i32, isOutput=False)
    ident_d = nc.declare_dram_parameter("ident", [128, 128], f32, isOutput=False)
    out = nc.declare_dram_parameter("out", [BC, 2], f32, isOutput=True)

    W = NSLOT // 16  # 640

    from contextlib import ExitStack
    with ExitStack() as ctx:
        sb = lambda name, shape, dt: ctx.enter_context(nc.sbuf_tensor(name, shape, dt))
        sem = lambda name: ctx.enter_context(nc.semaphore(name))

        kpw = sb("kpw_s", [128, W], i32)
        kpq = sb("kpq_s", [128, NC_], i32)
        kpf = sb("kpf_s", [NC_, 128], i32)
        bofs = sb("bofs_s", [128, W], i32)
        iot16 = sb("iot16_s", [128, 16 * NC_], i32)
        iq35 = sb("iq35", [BC, PAIR], i32)
        ip35 = sb("ip35", [BC, PAIR], i32)
        ident = sb("ident_s", [128, 128], f32)

        rt1 = sb("rt1", [128, W], i32)
        rt2 = sb("rt2", [128, W], i32)
        rf1 = sb("rf1", [128, W], f32)
        rf2 = sb("rf2", [128, W], f32)
        r16 = sb("r16", [128, W], i16)
        qv = sb("qv", [128, NC_], i32)

        big = sb("big", [128, NC_, ES], f32)
        oh = sb("oh", [128, 16 * NC_], f32)
        prodt = sb("prodt", [128, 16 * NC_], f32)
        val = sb("val", [128, NC_], f32)
        valT = sb("valT", [NC_, 128], f32)
        validf = sb("validf", [NC_, 128], f32)

        cnt = sb("cnt", [NC_, 7], f32)
        tv = sb("tv", [NC_, 119], f32)
        t2v = sb("t2v", [NC_, 119], f32)
        s1 = sb("s1", [NC_, 7], f32)
        s2 = sb("s2", [NC_, 7], f32)
        safe = sb("safe", [NC_, 7], f32)
        lnsafe = sb("lnsafe", [NC_, 7], f32)
        rsafe = sb("rsafe", [NC_, 7], f32)
        dpair = sb("dpair", [BC, 2], f32)
        lnd = sb("lnd", [BC, 2], f32)
        rdpair = sb("rdpair", [BC, 2], f32)
        rc = sb("rc", [NC_, 7], f32)
        means = sb("means", [NC_, 7], f32)
        m2 = sb("m2", [NC_, 7], f32)
        s2rc = sb("s2rc", [NC_, 7], f32)
        pull_p = sb("pull_p", [NC_, 7], f32)
        pv = sb("pv", [NC_, 7], f32)
        pack80 = sb("pack80", [NC_, 21], f32)
        pack16 = sb("pack16", [BC, NBLK * 21], f32)

        C = sb("C", [BC, 1], f32)
        pn_scr = sb("pn_scr", [BC, PP35], f32)
        pull_num = sb("pull_num", [BC, 1], f32)
        maxC = sb("maxC", [BC, 1], f32)
        rCm = sb("rCm", [BC, 1], f32)
        pull = sb("pull", [BC, 1], f32)
        diff = sb("diff", [BC, PAIR], f32)
        d2 = sb("d2", [BC, PAIR], f32)
        e = sb("e", [BC, PAIR], f32)
        tri35 = sb("tri35", [BC, PAIR], f32)
        iqf35 = sb("iqf35", [BC, PAIR], f32)
        mlt35 = sb("mlt35", [BC, PAIR], f32)
        em = sb("em", [BC, PAIR], f32)
        em2 = sb("em2", [BC, PAIR], f32)
        push_sum = sb("push_sum", [BC, 1], f32)
        Cm1 = sb("Cm1", [BC, 1], f32)
        CCt = sb("CCt", [BC, 1], f32)
        den = sb("den", [BC, 1], f32)
        rden = sb("rden", [BC, 1], f32)
        push = sb("push", [BC, 1], f32)
        outt = sb("outt", [BC, 2], f32)

        psumT = ctx.enter_context(nc.psum_tensor("psumT", [NC_, 128], f32))

        s_in0 = sem("s_in0")
        s_in = sem("s_in")
        sgq = [sem(f"sgq{q}") for q in range(4)]
        s_pe = sem("s_pe")
        s_rel = sem("s_rel")
        s_out = sem("s_out")
        vch = sem("vch")   # vector-op chain: op i incs to i+1 on completion
        ach = sem("ach")   # scalar-op chain
        block = ctx.enter_context(nc.Block())

        marks = {}

        @block.vector
        def _(vector):
            nv = [0]

            def V(fn, *a, mark=None, **k):
                # serialize DVE RAW hazards: wait for all prior vector ops
                vector.wait_ge(vch, nv[0])
                inst = fn(*a, **k).then_inc(vch, 1)
                nv[0] += 1
                if mark:
                    marks[mark] = nv[0]
                return inst

            vector.wait_ge(s_in0, 32)  # kpw + bofs loaded
            # r16 = (b%8)*4096 + (idx >> 4) as int16, wrapped layout
            V(vector.tensor_scalar, out=rt1[:, :], in0=kpw[:, :], scalar1=15,
              scalar2=None, op0=BAnd)
            V(vector.tensor_tensor, out=rt2[:, :], in0=kpw[:, :], in1=rt1[:, :], op=Sub)
            V(vector.tensor_copy, out=rf1[:, :], in_=rt2[:, :])
            V(vector.tensor_scalar, out=rf2[:, :], in0=rf1[:, :], scalar1=0.0625,
              scalar2=None, op0=Mult)
            V(vector.tensor_copy, out=rt1[:, :], in_=rf2[:, :])
            V(vector.tensor_tensor, out=rt2[:, :], in0=rt1[:, :], in1=bofs[:, :], op=Add)
            V(vector.tensor_copy, out=r16[:, :], in_=rt2[:, :], mark="r16")
            vector.wait_ge(s_in, 96)
            # independent prep while the gathers run
            V(vector.tensor_scalar, out=qv[:, :], in0=kpq[:, :], scalar1=15,
              scalar2=None, op0=BAnd)
            V(vector.tensor_copy, out=validf[:, :], in_=kpf[:, :])
            V(vector.tensor_tensor, out=tri35[:, :], in0=iq35[:, :], in1=ip35[:, :], op=IsGt)
            V(vector.tensor_copy, out=iqf35[:, :], in_=iq35[:, :])
            # one-hot select the target f32 out of each gathered 64-f32 row
            vector.wait_ge(sgq[0], 48)
            vector.wait_ge(sgq[1], 48)
            vector.wait_ge(sgq[2], 32)
            vector.wait_ge(sgq[3], 32)
            bigv = big[:, :, :].rearrange("p c (s f) -> p c s f", f=4)[:, :, :, 0]
            V(vector.tensor_tensor,
              out=oh[:, :].rearrange("p (c s) -> p c s", s=16),
              in0=iot16[:, :].rearrange("p (c s) -> p c s", s=16),
              in1=qv[:, :].unsqueeze(2).to_broadcast([128, NC_, 16]),
              op=IsEq)
            V(vector.tensor_tensor,
              out=prodt[:, :].rearrange("p (c s) -> p c s", s=16),
              in0=bigv,
              in1=oh[:, :].rearrange("p (c s) -> p c s", s=16),
              op=Mult)
            V(vector.tensor_reduce, out=val[:, :],
              in_=prodt[:, :].rearrange("p (c s) -> p c s", s=16),
              axis=X, op=Add, mark="val")
            # per-(b, block) person stats on the transposed values
            vector.wait_ge(s_pe, 1)
            V(vector.tensor_copy, out=valT[:, :], in_=psumT[:, :])
            vk = lambda tns, w: tns[:, :w].rearrange("c (p k) -> c p k", k=K)
            V(vector.tensor_reduce, out=cnt[:, :], in_=vk(validf, 119), axis=X, op=Add)
            V(vector.tensor_tensor, out=tv[:, :], in0=valT[:, :119],
              in1=validf[:, :119], op=Mult)
            V(vector.tensor_reduce, out=s1[:, :], in_=vk(tv, 119), axis=X, op=Add)
            V(vector.tensor_tensor, out=t2v[:, :], in0=tv[:, :], in1=valT[:, :119], op=Mult)
            V(vector.tensor_reduce, out=s2[:, :], in_=vk(t2v, 119), axis=X, op=Add)
            V(vector.tensor_scalar, out=safe[:, :], in0=cnt[:, :], scalar1=1.0,
              scalar2=None, op0=Max, mark="safe")
            V(vector.tensor_scalar, out=pv[:, :], in0=cnt[:, :], scalar1=0.0,
              scalar2=None, op0=IsGt)
            vector.wait_ge(ach, 2)   # rsafe ready
            V(vector.tensor_tensor, out=means[:, :], in0=s1[:, :], in1=rsafe[:, :], op=Mult)
            V(vector.tensor_tensor, out=m2[:, :], in0=means[:, :], in1=means[:, :], op=Mult)
            V(vector.tensor_tensor, out=s2rc[:, :], in0=s2[:, :], in1=rsafe[:, :], op=Mult)
            V(vector.tensor_tensor, out=pull_p[:, :], in0=s2rc[:, :], in1=m2[:, :], op=Sub)
            V(vector.tensor_copy, out=pack80[:, 0:7], in_=means[:, :])
            V(vector.tensor_copy, out=pack80[:, 7:14], in_=pull_p[:, :])
            V(vector.tensor_copy, out=pack80[:, 14:21], in_=pv[:, :], mark="pack")
            # batch-major finish
            vector.wait_ge(s_rel, 16)
            pk = pack16[:, :].rearrange("b (bl t p) -> b bl t p", bl=NBLK, t=3)
            meansv = pk[:, :, 0, :]   # [16, 5, 7]
            pullpv = pk[:, :, 1, :]
            pvv = pk[:, :, 2, :]
            V(vector.tensor_reduce, out=C[:, :], in_=pvv, axis=XY, op=Add)
            V(vector.tensor_scalar, out=dpair[:, 0:1], in0=C[:, :], scalar1=1.0,
              scalar2=None, op0=Max)
            V(vector.tensor_scalar, out=Cm1[:, :], in0=C[:, :], scalar1=1.0,
              scalar2=None, op0=Sub)
            V(vector.tensor_tensor, out=CCt[:, :], in0=C[:, :], in1=Cm1[:, :], op=Mult)
            V(vector.tensor_scalar, out=dpair[:, 1:2], in0=CCt[:, :], scalar1=2.0,
              scalar2=None, op0=Max, mark="dpair")
            V(vector.tensor_tensor,
              out=pn_scr[:, :].rearrange("b (bl p) -> b bl p", bl=NBLK),
              in0=pullpv, in1=pvv, op=Mult)
            V(vector.tensor_reduce, out=pull_num[:, :], in_=pn_scr[:, :], axis=X, op=Add)
            # diff[b, p1, p2] = means[p1] - means[p2]; DVE APs allow at most
            # 3 free dims, so emit one op per p1-block
            for qb in range(NBLK):
                V(vector.tensor_tensor,
                  out=diff[:, qb * 7 * PP35:(qb + 1) * 7 * PP35]
                      .rearrange("b (d q r) -> b d q r", d=7, q=NBLK),
                  in0=meansv[:, qb, :].unsqueeze(2).unsqueeze(3)
                      .to_broadcast([BC, 7, NBLK, 7]),
                  in1=meansv.unsqueeze(1).to_broadcast([BC, 7, NBLK, 7]),
                  op=Sub, mark="diff")
            V(vector.tensor_scalar, out=mlt35[:, :], in0=iqf35[:, :], scalar1=C[:, :1],
              scalar2=None, op0=IsLt)
            vector.wait_ge(ach, 4)   # rdpair ready
            V(vector.tensor_tensor, out=pull[:, :], in0=pull_num[:, :],
              in1=rdpair[:, 0:1], op=Mult)
            vector.wait_ge(ach, 6)   # e ready
            V(vector.tensor_tensor, out=em[:, :], in0=e[:, :], in1=mlt35[:, :], op=Mult)
            V(vector.tensor_tensor, out=em2[:, :], in0=em[:, :], in1=tri35[:, :], op=Mult)
            V(vector.tensor_reduce, out=push_sum[:, :], in_=em2[:, :], axis=X, op=Add)
            V(vector.tensor_tensor, out=push[:, :], in0=push_sum[:, :],
              in1=rdpair[:, 1:2], op=Mult)
            V(vector.tensor_copy, out=outt[:, 0:1], in_=push[:, :])
            V(vector.tensor_copy, out=outt[:, 1:2], in_=pull[:, :], mark="fin")

        @block.sync
        def _(sync):
            sync.dma_start(out=kpw[:, :], in_=kpw_d[:, :]).then_inc(s_in0, 16)
            sync.dma_start(out=bofs[:, :], in_=bofs_d[:, :]).then_inc(s_in0, 16)
            sync.dma_start(out=kpq[:, :], in_=kpq_d[:, :]).then_inc(s_in, 16)
            sync.dma_start(out=kpf[:, :], in_=kpf_d[:, :]).then_inc(s_in, 16)
            sync.dma_start(out=iot16[:, :], in_=iot16_d[:, :]).then_inc(s_in, 16)
            sync.dma_start(out=iq35[:, :], in_=iq_d[:, :]).then_inc(s_in, 16)
            sync.dma_start(out=ip35[:, :], in_=ip_d[:, :]).then_inc(s_in, 16)
            sync.dma_start(out=ident[:, :], in_=ident_d[:, :]).then_inc(s_in, 16)
            sync.wait_ge(vch, marks["pack"])
            # pack80 partitions iterate c = 5b+bl in exactly pack16's (b, bl)
            # order, so the whole relayout is one contiguous-partition DMA
            sync.dma_start(
                out=pack16[:, :].rearrange("b (bl i) -> b bl i", bl=NBLK),
                in_=pack80[:, :],
            ).then_inc(s_rel, 16)
            sync.wait_ge(vch, marks["fin"])
            sync.dma_start(out=out[:, :], in_=outt[:, :]).then_inc(s_out, 16)
            sync.wait_ge(s_out, 16)

        @block.gpsimd
        def _(gpsimd):
            gpsimd.wait_ge(vch, marks["r16"])
            # SWDGE gather ucode caps at 1024 idxs per instruction
            NCHUNK = 1024
            for h in range(2):
                src = tags[h * BC // 2:(h + 1) * BC // 2, :, :] \
                    .rearrange("b n d -> (b n d)").rearrange("(r e) -> r e", e=ES)
                for g in range(NHALF // NCHUNK):
                    j0 = g * NCHUNK
                    gpsimd.dma_gather(
                        big[:, h * 40 + j0 // 128:h * 40 + (j0 + NCHUNK) // 128, :],
                        src,
                        r16[:, h * (W // 2) + j0 // 16:
                            h * (W // 2) + (j0 + NCHUNK) // 16],
                        NCHUNK, NCHUNK, ES, queue_num=(h * 5 + g) % 4,
                    ).then_inc(sgq[(h * 5 + g) % 4], 16)

        @block.tensor
        def _(tensor):
            tensor.wait_ge(vch, marks["val"])
            tensor.transpose(out=psumT[:, :], in_=val[:, :],
                             identity=ident[:, :]).then_inc(s_pe, 1)

        @block.scalar
        def _(scalar):
            Ln = mybir.ActivationFunctionType.Ln
            Expf = mybir.ActivationFunctionType.Exp
            Sq = mybir.ActivationFunctionType.Square
            na = [0]

            def A(fn, *a, **k):
                scalar.wait_ge(ach, na[0])
                inst = fn(*a, **k).then_inc(ach, 1)
                na[0] += 1
                return inst

            scalar.wait_ge(vch, marks["safe"])
            A(scalar.activation, out=lnsafe[:, :], in_=safe[:, :], func=Ln)
            A(scalar.activation, out=rsafe[:, :], in_=lnsafe[:, :], func=Expf,
              scale=-1.0)
            scalar.wait_ge(vch, marks["dpair"])
            A(scalar.activation, out=lnd[:, :], in_=dpair[:, :], func=Ln)
            A(scalar.activation, out=rdpair[:, :], in_=lnd[:, :], func=Expf,
              scale=-1.0)
            scalar.wait_ge(vch, marks["diff"])
            A(scalar.activation, out=d2[:, :], in_=diff[:, :], func=Sq)
            A(scalar.activation, out=e[:, :], in_=d2[:, :], func=Expf,
              scale=-1.0)

    nc.compile()
    return nc


_CACHED_NC = None


def _get_nc() -> bass.Bass:
    global _CACHED_NC
    if _CACHED_NC is None:
        _CACHED_NC = build_nc()
    return _CACHED_NC


def make_in_maps(tags: np.ndarray, keypoints: np.ndarray) -> list[dict]:
    tags = np.ascontiguousarray(tags, dtype=np.float32)
    keypoints = np.ascontiguousarray(keypoints, dtype=np.int32)
    bofs, iot16, iq, ip, ident = host_consts()
    maps = []
    for i in range(NCORES):
        kpw, kpq, kpf = host_layouts(keypoints[i * BC:(i + 1) * BC])
        maps.append({
            "tags": tags[i * BC:(i + 1) * BC],
            "kpw": kpw, "kpq": kpq, "kpf": kpf,
            "bofs": bofs, "iot16": iot16, "iq": iq, "ip": ip, "ident": ident,
        })
    return maps


def kernel(tags: np.ndarray, keypoints: np.ndarray) -> np.ndarray:
    from concourse.bass_utils import run_bass_kernel_spmd

    nc = _get_nc()
    in_maps = make_in_maps(tags, keypoints)
    res = run_bass_kernel_spmd(nc, in_maps, core_ids=list(range(NCORES)))
    outs = [np.asarray(r["out"]) for r in res.results]
    return np.concatenate(outs, axis=0)



# revision 6
# speedup vs baseline: 1.0127x; 1.0127x over previous
"""Associative-embedding loss kernel for Trainium2, SPMD over 8 NeuronCores.

Inputs (full):
  tags:      [128, 65536, 4] float32
  keypoints: [128, 30, 17, 2] int32  (last dim = (heatmap_idx, valid_flag))
Output: [128, 2] float32 = stack([push, pull], -1)

Strategy: data parallel over the batch dim - 16 batches per core. Only
channel 0 of tags at 30*17 keypoint locations per batch is ever read, so
each core fetches the 256-byte rows containing its gather targets with two
5120-index GPSIMD dma_gather instructions (one per 8-batch half; the i16
gather index must address < 32768 rows, i.e. 8 batches). Gather row indices
arrive precomputed from the host as int16, and a host-built one-hot*valid
table turns the 16-candidate row select into a single multiply+reduce.

Slot layout (per half h): column c = 5*(b%8) + bl (bl = person-block), lane
u = 17*(p%7) + k; slot j = c*128 + u lands at big[u, 40h + c, :]. Per-person
sums (cnt, sum t, sum t^2) come from one fp32 matmul per half with a
stationary person-map [128x7], a PE transpose flips (person, column) ->
(column, person), and one relayout DMA yields batch-major stats [16, 105].
The final push/pull math is ~20 small DVE ops plus Square/Exp on the scalar
engine (both live in the exp_and_others table set, prewarmed during the
gather phase). Reciprocals run on the DVE, so no Ln/Exp table bouncing.
"""

import numpy as np

import concourse.bacc as bacc
import concourse.bass as bass
import concourse.mybir as mybir

B, N, D = 128, 65536, 4
P, K = 30, 17
NCORES = 8
BC = B // NCORES          # 16 batches per core
HB = 8                    # batches per half
NBLK = 5                  # person blocks per batch (7,7,7,7,2 persons)
NCH = NBLK * HB           # 40 columns per half
NC_ = 2 * NCH             # 80 columns per core
NHALF = NCH * 128         # 5120 gather slots per half
ES = 64                   # gathered row length (f32) = 256 bytes
NCHUNK = 1024             # gather idxs per dma_gather instruction
PP35 = NBLK * 7           # 35 padded person slots per batch
PAIR = PP35 * PP35        # 1225

f32 = mybir.dt.float32
i16 = mybir.dt.int16
Add = mybir.AluOpType.add
Sub = mybir.AluOpType.subtract
Mult = mybir.AluOpType.mult
Max = mybir.AluOpType.max
IsGt = mybir.AluOpType.is_gt
IsLt = mybir.AluOpType.is_lt
X = mybir.AxisListType.X


def host_layouts(kp_core: np.ndarray):
    """Host-side permutations of one core's keypoints [16,30,17,2]."""
    idx = kp_core[..., 0].astype(np.int64)                # [16,30,17]
    flg = (kp_core[..., 1] == 1).astype(np.float32)
    b = np.arange(BC)[:, None, None]
    pp = np.arange(P)[None, :, None]
    k = np.arange(K)[None, None, :]
    h = b // HB
    b8 = b % HB
    bl = pp // 7
    pidx = pp % 7
    ch = 5 * b8 + bl                                      # column within half
    u = 17 * pidx + k                                     # lane in column
    jh = ch * 128 + u                                     # slot within half
    bb, uu, jj, hh, cg = np.broadcast_arrays(b8, u, jh, h, h * NCH + ch)

    # int16 gather row indices, wrapped [16, n/16] and replicated to 128
    row = (bb * (N * D // ES) + (idx >> 4)).astype(np.int16)
    kpw = np.zeros((16, 2 * (NHALF // 16)), np.int16)
    kpw[jj % 16, (NHALF // 16) * hh + jj // 16] = row
    kpw = np.tile(kpw, (8, 1))                            # [128, 640]

    # one-hot (within-row position) pre-multiplied by the valid flag
    ohv = np.zeros((128, NC_ * 16), np.float32)
    ohv[uu, cg * 16 + (idx & 15)] = flg                   # [128, 1280]

    vf = np.zeros((128, NC_), np.float32)                 # valid flags, u-major
    vf[uu, cg] = flg
    return kpw, ohv, vf


def host_consts():
    """Input-independent tables."""
    pmap = np.zeros((128, 7), np.float32)                 # lane -> person map
    u = np.arange(7 * K)                                  # 119 real lanes
    pmap[u, u // K] = 1.0
    ident7 = np.eye(7, dtype=np.float32)
    q = np.arange(PP35)
    triq = np.tile((q[:, None] > q[None, :]).astype(np.float32).reshape(1, -1),
                   (BC, 1))                               # [16,1225] (q,p1): p1<q
    qiota = np.tile(q.astype(np.float32), (BC, 1))        # [16, 35]
    return pmap, ident7, triq, qiota


def build_nc(detect_race_conditions: bool = True) -> bass.Bass:
    nc = bacc.Bacc("TRN2", target_bir_lowering=False, debug=False,
                   num_swdge_queues=2,
                   detect_race_conditions=detect_race_conditions)
    tags = nc.declare_dram_parameter("tags", [BC, N, D], f32, isOutput=False)
    kpw_d = nc.declare_dram_parameter("kpw", [128, 2 * (NHALF // 16)], i16,
                                      isOutput=False)
    ohv_d = nc.declare_dram_parameter("ohv", [128, NC_ * 16], f32, isOutput=False)
    vf_d = nc.declare_dram_parameter("vf", [128, NC_], f32, isOutput=False)
    pmap_d = nc.declare_dram_parameter("pmap", [128, 7], f32, isOutput=False)
    id7_d = nc.declare_dram_parameter("id7", [7, 7], f32, isOutput=False)
    triq_d = nc.declare_dram_parameter("triq", [BC, PAIR], f32, isOutput=False)
    qio_d = nc.declare_dram_parameter("qio", [BC, PP35], f32, isOutput=False)
    out = nc.declare_dram_parameter("out", [BC, 2], f32, isOutput=True)

    W = 2 * (NHALF // 16)  # 640

    from contextlib import ExitStack
    with ExitStack() as ctx:
        sb = lambda name, shape, dt: ctx.enter_context(nc.sbuf_tensor(name, shape, dt))
        sem = lambda name: ctx.enter_context(nc.semaphore(name))

        kpw = sb("kpw_s", [128, W], i16)
        ohv = sb("ohv_s", [128, NC_ * 16], f32)
        pmap = sb("pmap_s", [128, 7], f32)
        id7 = sb("id7_s", [7, 7], f32)
        triq = sb("triq_s", [BC, PAIR], f32)
        qio = sb("qio_s", [BC, PP35], f32)

        big = sb("big", [128, NC_, ES], f32)
        prodt = sb("prodt", [128, 16 * NC_], f32)
        mstack = sb("mstack", [128, 240], f32)   # per half: [t*v | (t*v)^2 | v]
        scopy = [sb(f"scopy{h}", [7, 120], f32) for h in range(2)]
        sT = [sb(f"sT{h}", [120, 7], f32) for h in range(2)]
        stats16 = sb("stats16", [BC, 3 * PP35], f32)   # [s1 | s2 | cnt]

        safe16 = sb("safe16", [BC, PP35], f32)
        rsafe = sb("rsafe16", [BC, PP35], f32)
        means16 = sb("means16", [BC, PP35], f32)
        s2r = sb("s2r", [BC, PP35], f32)
        m2x = sb("m2x", [BC, PP35], f32)
        pullp = sb("pullp", [BC, PP35], f32)
        pv16 = sb("pv16", [BC, PP35], f32)
        pn16 = sb("pn16", [BC, PP35], f32)
        qm16 = sb("qm16", [BC, PP35], f32)
        rows16 = sb("rows16", [BC, PP35], f32)
        pr16 = sb("pr16", [BC, PP35], f32)
        C16 = sb("C16", [BC, 1], f32)
        pnum = sb("pnum", [BC, 1], f32)
        Cm1 = sb("Cm1", [BC, 1], f32)
        CCt = sb("CCt", [BC, 1], f32)
        psums = sb("psums", [BC, 1], f32)
        dpair = sb("dpair", [BC, 2], f32)
        rdp = sb("rdp", [BC, 2], f32)
        diffp = sb("diffp", [BC, PAIR], f32)
        d2 = sb("d2", [BC, PAIR], f32)
        ee = sb("ee", [BC, PAIR], f32)
        emt = sb("emt", [BC, PAIR], f32)
        outt = sb("outt", [BC, 2], f32)
        warm1 = sb("warm1", [1, 1], f32)

        ps1 = [ctx.enter_context(nc.psum_tensor(f"ps1{h}", [7, 120], f32))
               for h in range(2)]
        ps2 = [ctx.enter_context(nc.psum_tensor(f"ps2{h}", [120, 7], f32))
               for h in range(2)]

        s_kpw = sem("s_kpw")
        s_ohv = sem("s_ohv")
        s_vf = sem("s_vf")
        s_id = sem("s_id")
        s_tq = sem("s_tq")
        s_g = [sem("s_g0"), sem("s_g1")]
        s_rel = sem("s_rel")
        s_out = sem("s_out")
        vch = sem("vch")    # DVE op chain
        ach = sem("ach")    # scalar op chain
        pech = sem("pech")  # PE op chain
        block = ctx.enter_context(nc.Block())

        marks = {}

        @block.vector
        def _(vector):
            nv = [0]

            def V(fn, *a, mark=None, **k):
                vector.wait_ge(vch, nv[0])
                inst = fn(*a, **k).then_inc(vch, 1)
                nv[0] += 1
                if mark:
                    marks[mark] = nv[0]
                return inst

            vector.wait_ge(s_ohv, 16)
            for h in range(2):
                vector.wait_ge(s_g[h], 16 * (NHALF // NCHUNK))
                bigv = big[:, h * NCH:(h + 1) * NCH, :] \
                    .rearrange("p c (s f) -> p c s f", f=4)[:, :, :, 0]
                o16 = 16 * NCH
                V(vector.tensor_tensor,
                  out=prodt[:, h * o16:(h + 1) * o16]
                      .rearrange("p (c s) -> p c s", s=16),
                  in0=bigv,
                  in1=ohv[:, h * o16:(h + 1) * o16]
                      .rearrange("p (c s) -> p c s", s=16),
                  op=Mult)
                V(vector.tensor_reduce, out=mstack[:, h * 120:h * 120 + NCH],
                  in_=prodt[:, h * o16:(h + 1) * o16]
                      .rearrange("p (c s) -> p c s", s=16),
                  axis=X, op=Add)
                V(vector.tensor_tensor, out=mstack[:, h * 120 + NCH:h * 120 + 2 * NCH],
                  in0=mstack[:, h * 120:h * 120 + NCH],
                  in1=mstack[:, h * 120:h * 120 + NCH],
                  op=Mult, mark=f"mm{h}")
                vector.wait_ge(pech, 2 * h + 1)
                V(vector.tensor_copy, out=scopy[h][:, :], in_=ps1[h][:, :],
                  mark=f"sc{h}")
                vector.wait_ge(pech, 2 * h + 2)
                V(vector.tensor_copy, out=sT[h][:, :], in_=ps2[h][:, :],
                  mark=f"st{h}")

            # batch-major finish on [16, *]
            vector.wait_ge(s_rel, 96)
            vector.wait_ge(s_tq, 32)
            s1v = stats16[:, 0:PP35]
            s2v = stats16[:, PP35:2 * PP35]
            cntv = stats16[:, 2 * PP35:3 * PP35]
            V(vector.tensor_scalar, out=safe16[:, :], in0=cntv, scalar1=1.0,
              scalar2=None, op0=Max)
            V(vector.reciprocal, out=rsafe[:, :], in_=safe16[:, :])
            V(vector.tensor_tensor, out=means16[:, :], in0=s1v, in1=rsafe[:, :],
              op=Mult)
            V(vector.tensor_tensor,
              out=diffp[:, :].rearrange("b (q p) -> b q p", q=PP35),
              in0=means16[:, :].unsqueeze(2).to_broadcast([BC, PP35, PP35]),
              in1=means16[:, :].unsqueeze(1).to_broadcast([BC, PP35, PP35]),
              op=Sub, mark="dif")
            V(vector.tensor_scalar, out=pv16[:, :], in0=cntv, scalar1=0.0,
              scalar2=None, op0=IsGt)
            V(vector.tensor_reduce, out=C16[:, :], in_=pv16[:, :], axis=X, op=Add)
            V(vector.tensor_tensor, out=s2r[:, :], in0=s2v, in1=rsafe[:, :], op=Mult)
            V(vector.tensor_tensor, out=m2x[:, :], in0=means16[:, :],
              in1=means16[:, :], op=Mult)
            V(vector.tensor_tensor, out=pullp[:, :], in0=s2r[:, :], in1=m2x[:, :],
              op=Sub)
            V(vector.tensor_tensor, out=pn16[:, :], in0=pullp[:, :], in1=pv16[:, :],
              op=Mult)
            V(vector.tensor_reduce, out=pnum[:, :], in_=pn16[:, :], axis=X, op=Add)
            V(vector.tensor_scalar, out=Cm1[:, :], in0=C16[:, :], scalar1=1.0,
              scalar2=None, op0=Sub)
            V(vector.tensor_tensor, out=CCt[:, :], in0=C16[:, :], in1=Cm1[:, :],
              op=Mult)
            V(vector.tensor_scalar, out=dpair[:, 0:1], in0=C16[:, :], scalar1=1.0,
              scalar2=None, op0=Max)
            V(vector.tensor_scalar, out=dpair[:, 1:2], in0=CCt[:, :], scalar1=2.0,
              scalar2=None, op0=Max)
            V(vector.reciprocal, out=rdp[:, :], in_=dpair[:, :])
            V(vector.tensor_scalar, out=qm16[:, :], in0=qio[:, :],
              scalar1=C16[:, :1], scalar2=None, op0=IsLt)
            vector.wait_ge(ach, 3)
            V(vector.tensor_tensor, out=emt[:, :], in0=ee[:, :], in1=triq[:, :],
              op=Mult)
            V(vector.tensor_reduce, out=rows16[:, :],
              in_=emt[:, :].rearrange("b (q p) -> b q p", q=PP35), axis=X, op=Add)
            V(vector.tensor_tensor, out=pr16[:, :], in0=rows16[:, :],
              in1=qm16[:, :], op=Mult)
            V(vector.tensor_reduce, out=psums[:, :], in_=pr16[:, :], axis=X, op=Add)
            V(vector.tensor_tensor, out=outt[:, 0:1], in0=psums[:, :],
              in1=rdp[:, 1:2], op=Mult)
            V(vector.tensor_tensor, out=outt[:, 1:2], in0=pnum[:, :],
              in1=rdp[:, 0:1], op=Mult, mark="fin")

        @block.scalar
        def _(scalar):
            Expf = mybir.ActivationFunctionType.Exp
            Sq = mybir.ActivationFunctionType.Square
            na = [0]

            def A(fn, *a, **k):
                scalar.wait_ge(ach, na[0])
                inst = fn(*a, **k).then_inc(ach, 1)
                na[0] += 1
                return inst

            # pull the exp_and_others table in early, while the gathers run
            scalar.wait_ge(s_id, 16)
            A(scalar.activation, out=warm1[:, :], in_=id7[0:1, 0:1], func=Expf)
            scalar.wait_ge(vch, marks["dif"])
            A(scalar.activation, out=d2[:, :], in_=diffp[:, :], func=Sq)
            A(scalar.activation, out=ee[:, :], in_=d2[:, :], func=Expf,
              scale=-1.0)

        @block.tensor
        def _(tensor):
            tensor.wait_ge(s_vf, 48)
            tensor.wait_ge(s_id, 16)
            for h in range(2):
                tensor.wait_ge(vch, marks[f"mm{h}"])
                tensor.matmul(ps1[h][:, :], pmap[:, :],
                              mstack[:, h * 120:(h + 1) * 120]).then_inc(pech, 1)
                tensor.wait_ge(vch, marks[f"sc{h}"])
                tensor.transpose(out=ps2[h][:, :], in_=scopy[h][:, :],
                                 identity=id7[:, :]).then_inc(pech, 1)

        @block.gpsimd
        def _(gpsimd):
            gpsimd.wait_ge(s_kpw, 16)
            for h in range(2):
                src = tags[h * HB:(h + 1) * HB, :, :] \
                    .rearrange("b n d -> (b n d)").rearrange("(r e) -> r e", e=ES)
                for g in range(NHALF // NCHUNK):
                    j0 = g * NCHUNK
                    gpsimd.dma_gather(
                        big[:, h * NCH + j0 // 128:h * NCH + (j0 + NCHUNK) // 128, :],
                        src,
                        kpw[:, h * (W // 2) + j0 // 16:
                            h * (W // 2) + (j0 + NCHUNK) // 16],
                        NCHUNK, NCHUNK, ES, queue_num=h,
                    ).then_inc(s_g[h], 16)

        @block.sync
        def _(sync):
            sync.dma_start(out=kpw[:, :], in_=kpw_d[:, :]).then_inc(s_kpw, 16)
            sync.dma_start(out=ohv[:, :], in_=ohv_d[:, :]).then_inc(s_ohv, 16)
            sync.dma_start(out=mstack[:, 80:120], in_=vf_d[:, 0:NCH]).then_inc(s_vf, 16)
            sync.dma_start(out=mstack[:, 200:240], in_=vf_d[:, NCH:NC_]).then_inc(s_vf, 16)
            sync.dma_start(out=pmap[:, :], in_=pmap_d[:, :]).then_inc(s_vf, 16)
            sync.dma_start(out=id7[:, :], in_=id7_d[:, :]).then_inc(s_id, 16)
            sync.dma_start(out=triq[:, :], in_=triq_d[:, :]).then_inc(s_tq, 16)
            sync.dma_start(out=qio[:, :], in_=qio_d[:, :]).then_inc(s_tq, 16)
            for h in range(2):
                sync.wait_ge(vch, marks[f"st{h}"])
                for s in range(3):
                    sync.dma_start(
                        out=stats16[h * HB:(h + 1) * HB, s * PP35:(s + 1) * PP35]
                            .rearrange("b (bl j) -> b bl j", j=7),
                        in_=sT[h][s * NCH:(s + 1) * NCH, :],
                    ).then_inc(s_rel, 16)
            sync.wait_ge(vch, marks["fin"])
            sync.dma_start(out=out[:, :], in_=outt[:, :]).then_inc(s_out, 16)
            sync.wait_ge(s_out, 16)

    nc.compile()
    return nc


_CACHED_NC = None


def _get_nc() -> bass.Bass:
    global _CACHED_NC
    if _CACHED_NC is None:
        _CACHED_NC = build_nc()
    return _CACHED_NC


def make_in_maps(tags: np.ndarray, keypoints: np.ndarray) -> list[dict]:
    tags = np.ascontiguousarray(tags, dtype=np.float32)
    keypoints = np.ascontiguousarray(keypoints, dtype=np.int32)
    pmap, ident7, triq, qiota = host_consts()
    maps = []
    for i in range(NCORES):
        kpw, ohv, vf = host_layouts(keypoints[i * BC:(i + 1) * BC])
        maps.append({
            "tags": tags[i * BC:(i + 1) * BC],
            "kpw": kpw, "ohv": ohv, "vf": vf,
            "pmap": pmap, "id7": ident7, "triq": triq, "qio": qiota,
        })
    return maps


def kernel(tags: np.ndarray, keypoints: np.ndarray) -> np.ndarray:
    from concourse.bass_utils import run_bass_kernel_spmd

    nc = _get_nc()
    in_maps = make_in_maps(tags, keypoints)
    res = run_bass_kernel_spmd(nc, in_maps, core_ids=list(range(NCORES)))
    outs = [np.asarray(r["out"]) for r in res.results]
    return np.concatenate(outs, axis=0)


# revision 7
# speedup vs baseline: 1.2266x; 1.2112x over previous
"""Associative-embedding loss kernel for Trainium2, SPMD over 8 NeuronCores.

Inputs (full):
  tags:      [128, 65536, 4] float32
  keypoints: [128, 30, 17, 2] int32  (last dim = (heatmap_idx, valid_flag))
Output: [128, 2] float32 = stack([push, pull], -1)

Strategy: data parallel over the batch dim - 16 batches per core. Only
channel 0 of tags at 30*17 keypoint locations per batch is ever read, so
each core fetches the 256-byte rows containing its gather targets with two
5120-index GPSIMD dma_gather instructions (one per 8-batch half; the i16
gather index must address < 32768 rows, i.e. 8 batches). Gather row indices
arrive precomputed from the host as int16, and a host-built one-hot*valid
table turns the 16-candidate row select into a single multiply+reduce.

Slot layout (per half h): column c = 5*(b%8) + bl (bl = person-block), lane
u = 17*(p%7) + k; slot j = c*128 + u lands at big[u, 40h + c, :]. Per-person
sums (cnt, sum t, sum t^2) come from one fp32 matmul per half with a
stationary person-map [128x7], a PE transpose flips (person, column) ->
(column, person), and one relayout DMA yields batch-major stats [16, 105].
The final push/pull math is ~20 small DVE ops plus Square/Exp on the scalar
engine (both live in the exp_and_others table set, prewarmed during the
gather phase). Reciprocals run on the DVE, so no Ln/Exp table bouncing.
"""

import numpy as np

import concourse.bacc as bacc
import concourse.bass as bass
import concourse.mybir as mybir

B, N, D = 128, 65536, 4
P, K = 30, 17
NCORES = 8
BC = B // NCORES          # 16 batches per core
HB = 8                    # batches per half
NBLK = 5                  # person blocks per batch (7,7,7,7,2 persons)
NCH = NBLK * HB           # 40 columns per half
NC_ = 2 * NCH             # 80 columns per core
NHALF = NCH * 128         # 5120 gather slots per half
ES = 64                   # gathered row length (f32) = 256 bytes
NCHUNK = 1024             # gather idxs per dma_gather instruction
PP35 = NBLK * 7           # 35 padded person slots per batch
PAIR = PP35 * PP35        # 1225

f32 = mybir.dt.float32
i16 = mybir.dt.int16
Add = mybir.AluOpType.add
Sub = mybir.AluOpType.subtract
Mult = mybir.AluOpType.mult
Max = mybir.AluOpType.max
IsGt = mybir.AluOpType.is_gt
IsLt = mybir.AluOpType.is_lt
X = mybir.AxisListType.X


def host_layouts(kp_core: np.ndarray):
    """Host-side permutations of one core's keypoints [16,30,17,2]."""
    idx = kp_core[..., 0].astype(np.int64)                # [16,30,17]
    flg = (kp_core[..., 1] == 1).astype(np.float32)
    b = np.arange(BC)[:, None, None]
    pp = np.arange(P)[None, :, None]
    k = np.arange(K)[None, None, :]
    h = b // HB
    b8 = b % HB
    bl = pp // 7
    pidx = pp % 7
    ch = 5 * b8 + bl                                      # column within half
    u = 17 * pidx + k                                     # lane in column
    jh = ch * 128 + u                                     # slot within half
    bb, uu, jj, hh, cg = np.broadcast_arrays(b8, u, jh, h, h * NCH + ch)

    # int16 gather row indices, wrapped [16, n/16] and replicated to 128
    row = (bb * (N * D // ES) + (idx >> 4)).astype(np.int16)
    kpw = np.zeros((16, 2 * (NHALF // 16)), np.int16)
    kpw[jj % 16, (NHALF // 16) * hh + jj // 16] = row
    kpw = np.tile(kpw, (8, 1))                            # [128, 640]

    # one-hot (within-row position) pre-multiplied by the valid flag
    ohv = np.zeros((128, NC_ * 16), np.float32)
    ohv[uu, cg * 16 + (idx & 15)] = flg                   # [128, 1280]

    vf = np.zeros((128, NC_), np.float32)                 # valid flags, u-major
    vf[uu, cg] = flg
    return kpw, ohv, vf


def host_consts():
    """Input-independent tables."""
    pmap = np.zeros((128, 7), np.float32)                 # lane -> person map
    u = np.arange(7 * K)                                  # 119 real lanes
    pmap[u, u // K] = 1.0
    ident7 = np.eye(7, dtype=np.float32)
    q = np.arange(PP35)
    triq = np.tile((q[:, None] > q[None, :]).astype(np.float32).reshape(1, -1),
                   (BC, 1))                               # [16,1225] (q,p1): p1<q
    qiota = np.tile(q.astype(np.float32), (BC, 1))        # [16, 35]
    return pmap, ident7, triq, qiota


def build_nc(detect_race_conditions: bool = True) -> bass.Bass:
    nc = bacc.Bacc("TRN2", target_bir_lowering=False, debug=False,
                   num_swdge_queues=4,
                   detect_race_conditions=detect_race_conditions)
    tags = nc.declare_dram_parameter("tags", [BC, N, D], f32, isOutput=False)
    kpw_d = nc.declare_dram_parameter("kpw", [128, 2 * (NHALF // 16)], i16,
                                      isOutput=False)
    ohv_d = nc.declare_dram_parameter("ohv", [128, NC_ * 16], f32, isOutput=False)
    vf_d = nc.declare_dram_parameter("vf", [128, NC_], f32, isOutput=False)
    pmap_d = nc.declare_dram_parameter("pmap", [128, 7], f32, isOutput=False)
    id7_d = nc.declare_dram_parameter("id7", [7, 7], f32, isOutput=False)
    triq_d = nc.declare_dram_parameter("triq", [BC, PAIR], f32, isOutput=False)
    qio_d = nc.declare_dram_parameter("qio", [BC, PP35], f32, isOutput=False)
    out = nc.declare_dram_parameter("out", [BC, 2], f32, isOutput=True)

    W = 2 * (NHALF // 16)  # 640

    from contextlib import ExitStack
    with ExitStack() as ctx:
        sb = lambda name, shape, dt: ctx.enter_context(nc.sbuf_tensor(name, shape, dt))
        sem = lambda name: ctx.enter_context(nc.semaphore(name))

        kpw = sb("kpw_s", [128, W], i16)
        ohv = sb("ohv_s", [128, NC_ * 16], f32)
        pmap = sb("pmap_s", [128, 7], f32)
        id7 = sb("id7_s", [7, 7], f32)
        triq = sb("triq_s", [BC, PAIR], f32)
        qio = sb("qio_s", [BC, PP35], f32)

        big = sb("big", [128, NC_, ES], f32)
        prodt = sb("prodt", [128, 16 * NC_], f32)
        mstack = sb("mstack", [128, 240], f32)   # per half: [t*v | (t*v)^2 | v]
        scopy = [sb(f"scopy{h}", [7, 120], f32) for h in range(2)]
        sT = [sb(f"sT{h}", [120, 7], f32) for h in range(2)]
        stats16 = sb("stats16", [BC, 3 * PP35], f32)   # [s1 | s2 | cnt]

        safe16 = sb("safe16", [BC, PP35], f32)
        rsafe = sb("rsafe16", [BC, PP35], f32)
        means16 = sb("means16", [BC, PP35], f32)
        s2r = sb("s2r", [BC, PP35], f32)
        m2x = sb("m2x", [BC, PP35], f32)
        pullp = sb("pullp", [BC, PP35], f32)
        pv16 = sb("pv16", [BC, PP35], f32)
        pn16 = sb("pn16", [BC, PP35], f32)
        qm16 = sb("qm16", [BC, PP35], f32)
        rows16 = sb("rows16", [BC, PP35], f32)
        pr16 = sb("pr16", [BC, PP35], f32)
        C16 = sb("C16", [BC, 1], f32)
        pnum = sb("pnum", [BC, 1], f32)
        Cm1 = sb("Cm1", [BC, 1], f32)
        CCt = sb("CCt", [BC, 1], f32)
        psums = sb("psums", [BC, 1], f32)
        dpair = sb("dpair", [BC, 2], f32)
        rdp = sb("rdp", [BC, 2], f32)
        diffp = sb("diffp", [BC, PAIR], f32)
        d2 = sb("d2", [BC, PAIR], f32)
        ee = sb("ee", [BC, PAIR], f32)
        emt = sb("emt", [BC, PAIR], f32)
        outt = sb("outt", [BC, 2], f32)
        warm1 = sb("warm1", [1, 1], f32)

        ps1 = [ctx.enter_context(nc.psum_tensor(f"ps1{h}", [7, 120], f32))
               for h in range(2)]
        ps2 = [ctx.enter_context(nc.psum_tensor(f"ps2{h}", [120, 7], f32))
               for h in range(2)]

        s_kpw = sem("s_kpw")
        s_ohv = sem("s_ohv")
        s_vf = sem("s_vf")
        s_id = sem("s_id")
        s_tq = sem("s_tq")
        s_g = [sem("s_g0"), sem("s_g1")]
        s_rel = sem("s_rel")
        s_out = sem("s_out")
        vch = sem("vch")    # DVE op chain
        ach = sem("ach")    # scalar op chain
        pech = sem("pech")  # PE op chain
        block = ctx.enter_context(nc.Block())

        marks = {}

        @block.vector
        def _(vector):
            nv = [0]

            def V(fn, *a, mark=None, **k):
                vector.wait_ge(vch, nv[0])
                inst = fn(*a, **k).then_inc(vch, 1)
                nv[0] += 1
                if mark:
                    marks[mark] = nv[0]
                return inst

            vector.wait_ge(s_ohv, 16)
            for h in range(2):
                vector.wait_ge(s_g[h], 16 * (NHALF // NCHUNK))
                bigv = big[:, h * NCH:(h + 1) * NCH, :] \
                    .rearrange("p c (s f) -> p c s f", f=4)[:, :, :, 0]
                o16 = 16 * NCH
                V(vector.tensor_tensor,
                  out=prodt[:, h * o16:(h + 1) * o16]
                      .rearrange("p (c s) -> p c s", s=16),
                  in0=bigv,
                  in1=ohv[:, h * o16:(h + 1) * o16]
                      .rearrange("p (c s) -> p c s", s=16),
                  op=Mult)
                V(vector.tensor_reduce, out=mstack[:, h * 120:h * 120 + NCH],
                  in_=prodt[:, h * o16:(h + 1) * o16]
                      .rearrange("p (c s) -> p c s", s=16),
                  axis=X, op=Add)
                V(vector.tensor_tensor, out=mstack[:, h * 120 + NCH:h * 120 + 2 * NCH],
                  in0=mstack[:, h * 120:h * 120 + NCH],
                  in1=mstack[:, h * 120:h * 120 + NCH],
                  op=Mult, mark=f"mm{h}")
                vector.wait_ge(pech, 2 * h + 1)
                V(vector.tensor_copy, out=scopy[h][:, :], in_=ps1[h][:, :],
                  mark=f"sc{h}")
                vector.wait_ge(pech, 2 * h + 2)
                V(vector.tensor_copy, out=sT[h][:, :], in_=ps2[h][:, :],
                  mark=f"st{h}")

            # batch-major finish on [16, *]
            vector.wait_ge(s_rel, 96)
            vector.wait_ge(s_tq, 32)
            s1v = stats16[:, 0:PP35]
            s2v = stats16[:, PP35:2 * PP35]
            cntv = stats16[:, 2 * PP35:3 * PP35]
            V(vector.tensor_scalar, out=safe16[:, :], in0=cntv, scalar1=1.0,
              scalar2=None, op0=Max)
            V(vector.reciprocal, out=rsafe[:, :], in_=safe16[:, :])
            V(vector.tensor_tensor, out=means16[:, :], in0=s1v, in1=rsafe[:, :],
              op=Mult)
            V(vector.tensor_tensor,
              out=diffp[:, :].rearrange("b (q p) -> b q p", q=PP35),
              in0=means16[:, :].unsqueeze(2).to_broadcast([BC, PP35, PP35]),
              in1=means16[:, :].unsqueeze(1).to_broadcast([BC, PP35, PP35]),
              op=Sub, mark="dif")
            V(vector.tensor_scalar, out=pv16[:, :], in0=cntv, scalar1=0.0,
              scalar2=None, op0=IsGt)
            V(vector.tensor_reduce, out=C16[:, :], in_=pv16[:, :], axis=X, op=Add)
            V(vector.tensor_tensor, out=s2r[:, :], in0=s2v, in1=rsafe[:, :], op=Mult)
            V(vector.tensor_tensor, out=m2x[:, :], in0=means16[:, :],
              in1=means16[:, :], op=Mult)
            V(vector.tensor_tensor, out=pullp[:, :], in0=s2r[:, :], in1=m2x[:, :],
              op=Sub)
            V(vector.tensor_tensor, out=pn16[:, :], in0=pullp[:, :], in1=pv16[:, :],
              op=Mult)
            V(vector.tensor_reduce, out=pnum[:, :], in_=pn16[:, :], axis=X, op=Add)
            V(vector.tensor_scalar, out=Cm1[:, :], in0=C16[:, :], scalar1=1.0,
              scalar2=None, op0=Sub)
            V(vector.tensor_tensor, out=CCt[:, :], in0=C16[:, :], in1=Cm1[:, :],
              op=Mult)
            V(vector.tensor_scalar, out=dpair[:, 0:1], in0=C16[:, :], scalar1=1.0,
              scalar2=None, op0=Max)
            V(vector.tensor_scalar, out=dpair[:, 1:2], in0=CCt[:, :], scalar1=2.0,
              scalar2=None, op0=Max)
            V(vector.reciprocal, out=rdp[:, :], in_=dpair[:, :])
            V(vector.tensor_scalar, out=qm16[:, :], in0=qio[:, :],
              scalar1=C16[:, :1], scalar2=None, op0=IsLt)
            vector.wait_ge(ach, 3)
            V(vector.tensor_tensor, out=emt[:, :], in0=ee[:, :], in1=triq[:, :],
              op=Mult)
            V(vector.tensor_reduce, out=rows16[:, :],
              in_=emt[:, :].rearrange("b (q p) -> b q p", q=PP35), axis=X, op=Add)
            V(vector.tensor_tensor, out=pr16[:, :], in0=rows16[:, :],
              in1=qm16[:, :], op=Mult)
            V(vector.tensor_reduce, out=psums[:, :], in_=pr16[:, :], axis=X, op=Add)
            V(vector.tensor_tensor, out=outt[:, 0:1], in0=psums[:, :],
              in1=rdp[:, 1:2], op=Mult)
            V(vector.tensor_tensor, out=outt[:, 1:2], in0=pnum[:, :],
              in1=rdp[:, 0:1], op=Mult, mark="fin")

        @block.scalar
        def _(scalar):
            Expf = mybir.ActivationFunctionType.Exp
            Sq = mybir.ActivationFunctionType.Square
            na = [0]

            def A(fn, *a, **k):
                scalar.wait_ge(ach, na[0])
                inst = fn(*a, **k).then_inc(ach, 1)
                na[0] += 1
                return inst

            # pull the exp_and_others table in early, while the gathers run
            scalar.wait_ge(s_id, 16)
            A(scalar.activation, out=warm1[:, :], in_=id7[0:1, 0:1], func=Expf)
            scalar.wait_ge(vch, marks["dif"])
            A(scalar.activation, out=d2[:, :], in_=diffp[:, :], func=Sq)
            A(scalar.activation, out=ee[:, :], in_=d2[:, :], func=Expf,
              scale=-1.0)

        @block.tensor
        def _(tensor):
            tensor.wait_ge(s_vf, 48)
            tensor.wait_ge(s_id, 16)
            for h in range(2):
                tensor.wait_ge(vch, marks[f"mm{h}"])
                tensor.matmul(ps1[h][:, :], pmap[:, :],
                              mstack[:, h * 120:(h + 1) * 120]).then_inc(pech, 1)
                tensor.wait_ge(vch, marks[f"sc{h}"])
                tensor.transpose(out=ps2[h][:, :], in_=scopy[h][:, :],
                                 identity=id7[:, :]).then_inc(pech, 1)

        @block.gpsimd
        def _(gpsimd):
            gpsimd.wait_ge(s_kpw, 16)
            for h in range(2):
                src = tags[h * HB:(h + 1) * HB, :, :] \
                    .rearrange("b n d -> (b n d)").rearrange("(r e) -> r e", e=ES)
                for g in range(NHALF // NCHUNK):
                    j0 = g * NCHUNK
                    gpsimd.dma_gather(
                        big[:, h * NCH + j0 // 128:h * NCH + (j0 + NCHUNK) // 128, :],
                        src,
                        kpw[:, h * (W // 2) + j0 // 16:
                            h * (W // 2) + (j0 + NCHUNK) // 16],
                        NCHUNK, NCHUNK, ES,
                        queue_num=(h * (NHALF // NCHUNK) + g) % 4,
                    ).then_inc(s_g[h], 16)

        @block.sync
        def _(sync):
            sync.dma_start(out=kpw[:, :], in_=kpw_d[:, :]).then_inc(s_kpw, 16)
            sync.dma_start(out=ohv[:, :], in_=ohv_d[:, :]).then_inc(s_ohv, 16)
            sync.dma_start(out=mstack[:, 80:120], in_=vf_d[:, 0:NCH]).then_inc(s_vf, 16)
            sync.dma_start(out=mstack[:, 200:240], in_=vf_d[:, NCH:NC_]).then_inc(s_vf, 16)
            sync.dma_start(out=pmap[:, :], in_=pmap_d[:, :]).then_inc(s_vf, 16)
            sync.dma_start(out=id7[:, :], in_=id7_d[:, :]).then_inc(s_id, 16)
            sync.dma_start(out=triq[:, :], in_=triq_d[:, :]).then_inc(s_tq, 16)
            sync.dma_start(out=qio[:, :], in_=qio_d[:, :]).then_inc(s_tq, 16)
            for h in range(2):
                sync.wait_ge(vch, marks[f"st{h}"])
                for s in range(3):
                    sync.dma_start(
                        out=stats16[h * HB:(h + 1) * HB, s * PP35:(s + 1) * PP35]
                            .rearrange("b (bl j) -> b bl j", j=7),
                        in_=sT[h][s * NCH:(s + 1) * NCH, :],
                    ).then_inc(s_rel, 16)
            sync.wait_ge(vch, marks["fin"])
            sync.dma_start(out=out[:, :], in_=outt[:, :]).then_inc(s_out, 16)
            sync.wait_ge(s_out, 16)

    nc.compile()
    return nc


_CACHED_NC = None


def _get_nc() -> bass.Bass:
    global _CACHED_NC
    if _CACHED_NC is None:
        _CACHED_NC = build_nc()
    return _CACHED_NC


def make_in_maps(tags: np.ndarray, keypoints: np.ndarray) -> list[dict]:
    tags = np.ascontiguousarray(tags, dtype=np.float32)
    keypoints = np.ascontiguousarray(keypoints, dtype=np.int32)
    pmap, ident7, triq, qiota = host_consts()
    maps = []
    for i in range(NCORES):
        kpw, ohv, vf = host_layouts(keypoints[i * BC:(i + 1) * BC])
        maps.append({
            "tags": tags[i * BC:(i + 1) * BC],
            "kpw": kpw, "ohv": ohv, "vf": vf,
            "pmap": pmap, "id7": ident7, "triq": triq, "qio": qiota,
        })
    return maps


def kernel(tags: np.ndarray, keypoints: np.ndarray) -> np.ndarray:
    from concourse.bass_utils import run_bass_kernel_spmd

    nc = _get_nc()
    in_maps = make_in_maps(tags, keypoints)
    res = run_bass_kernel_spmd(nc, in_maps, core_ids=list(range(NCORES)))
    outs = [np.asarray(r["out"]) for r in res.results]
    return np.concatenate(outs, axis=0)
